# revision 13
# baseline (speedup 1.0000x reference)
"""Cross-attention kernel for Trainium2 (8 NeuronCores, Bass/Tile).

Problem: nn_CrossAttention — B=4, C=256, H=W=64 (N=4096 tokens), CI=128.
  q = q_w @ x + q_b            [B, N, CI]
  k = k_w @ rgbd + k_b         [B, CI, N]
  v = v_w @ rgbd + v_b         [B, N, CI]
  out = rgbd + out_w @ (softmax(q k) v) + out_b

Sharding: data-parallel over batch x query-half. Core i handles batch i//2,
query half i%2 (2048 queries, all 4096 keys). No collectives needed.

Math simplifications (exact):
  - k_b drops out of softmax (adds a per-query constant to logits).
  - v_b commutes with the softmax average -> fused output bias (host).
  - exp() without max-subtraction: logits are bounded (|S| < ~30), safe fp32.

Perf design (vs. the 133us baseline):
  - PE p-state: the PE reaches 2.4GHz only after ~3us of CONTINUOUS busy;
    any idle gap resets it to 1.2GHz.  Everything is arranged to keep the
    PE streaming without gaps: warm-up dummies bridge the input-DMA wait,
    and no consumer chain ever blocks the PE queue.
  - exp is split between ScalarE (ACT table exp) and VectorE (Schraudolph
    bitcast exp: et = bf16_bits(int16(A*s + B)), ~3% rel err that cancels
    in softmax since the denominator sums the same approximated values).
  - No ACT table swaps: ScalarE runs ONLY Exp/Identity (one table load).
    1/d uses DVE reciprocal_approx_fast (single custom-DVE op).
  - V^T computed directly (rs chunks as stationary weights) - no PE
    transposes, casts ride the idle GpSimd/Pool engine.
  - Host pre-swizzles inputs to the exact SBUF layouts so every DMA is a
    simple descriptor; DMAs are issued from 4 different engine queues in
    parallel right at kernel start.  Residual comes from the bf16 rs/resb
    copies (no separate f32 residual load).
  - Normalization is applied after the output projection, so the final
    matmuls never wait on the reciprocal; tail elementwise work is spread
    over Pool and DVE.
"""

import numpy as np

B, C, HH, WW = 4, 256, 64, 64
CI = 128
N = HH * WW            # 4096 tokens per batch
NCORES = 8
QSH = N // 2           # 2048 queries per core
QCH = 512              # query chunk (matmul moving free dim)
NQC = QSH // QCH       # 4 q-chunks
NKC = N // 128         # 32 key chunks of 128
EB = 2                 # S tiles per exp batch
KQ = 8                 # key chunks per et quarter-buffer
NQUARTER = NKC // KQ   # 4 quarters
NDUMMY = 34            # PE warm-up matmuls (128-free each)

# Schraudolph exp->bf16 constants: I = trunc(A*s + B) as int16, bitcast bf16.
# A = 128/ln2; B centers the log-error band (+0.5 for truncation).
SCH_A = 184.66502305344
SCH_B = 16250.9914
# per-quarter exp engine pattern (True -> DVE Schraudolph), 10 ACT / 6 DVE
DVE_PAT = [
    (False, True, False, True),
    (False, True, False, False),
    (False, True, False, False),
    (False, True, False, True),
]

_CACHE = {}


def _build_nc():
    import concourse.bass as bass
    import concourse.mybir as mybir
    import concourse.tile as tile
    from concourse import bacc
    from concourse.bass import ts

    f32 = mybir.dt.float32
    f32r = mybir.dt.float32r
    bf16 = mybir.dt.bfloat16
    i16 = mybir.dt.int16
    EXP = mybir.ActivationFunctionType.Exp
    IDENT = mybir.ActivationFunctionType.Identity
    ADD = mybir.AluOpType.add
    MULT = mybir.AluOpType.mult

    nc = bacc.Bacc("TRN2", target_bir_lowering=False, debug=False)

    rs_d = nc.dram_tensor("rs", [128, 2, N], bf16, kind="ExternalInput")
    xs_d = nc.dram_tensor("xs", [128, 2, QSH], bf16, kind="ExternalInput")
    resob_d = nc.dram_tensor("resob", [128, 2, QSH], bf16, kind="ExternalInput")
    wb_d = nc.dram_tensor("wb", [128, 2, 384], bf16, kind="ExternalInput")
    wf_d = nc.dram_tensor("wf", [128, 257], f32r, kind="ExternalInput")
    out_d = nc.dram_tensor("out", [128, 2, QSH], f32, kind="ExternalOutput")

    with tile.TileContext(nc) as tc:
        with (
            tc.tile_pool(name="const", bufs=1) as cpool,
            tc.tile_pool(name="big", bufs=3) as bigpool,
            tc.tile_pool(name="work", bufs=2) as wpool,
            tc.tile_pool(name="ps_s", bufs=2, space=bass.MemorySpace.PSUM) as ps_s,
            tc.tile_pool(name="ps_d", bufs=1, space=bass.MemorySpace.PSUM) as ps_d,
            tc.tile_pool(name="ps_o", bufs=2, space=bass.MemorySpace.PSUM) as ps_o,
            tc.tile_pool(name="ps_t", bufs=1, space=bass.MemorySpace.PSUM) as ps_t,
        ):
            # ---- SBUF tiles ----
            wb_sb = cpool.tile([128, 2, 384], bf16, tag="wb")
            wf_sb = cpool.tile([128, 257], f32r, tag="wf")
            rs_sb = cpool.tile([128, 2, N], bf16, tag="rs")
            xs_sb = cpool.tile([128, 2, QSH], bf16, tag="xs")
            resob_sb = cpool.tile([128, 2, QSH], bf16, tag="resob")
            K_sb = cpool.tile([128, N], bf16, tag="K")
            QT_sb = cpool.tile([128, QSH], bf16, tag="QT")
            V_sb = cpool.tile([128, NKC, 128], bf16, tag="V")
            ones32 = cpool.tile([128, 32], bf16, tag="ones32")
            onesR = cpool.tile([128, 128], f32r, tag="onesR")
            wsrc = cpool.tile([128, 128], bf16, tag="wsrc")

            qw = wb_sb[:, :, 0:128]
            kw = wb_sb[:, :, 128:256]
            vw2 = wb_sb[:, :, 256:384]
            ow = wf_sb[:, 0:256]
            qb = wf_sb[:, 256:257].bitcast(f32)

            # reshaped views for wide casts
            K_r = K_sb[:].rearrange("p (j i n) -> p j i n", i=2, n=QCH)
            QT_r = QT_sb[:].rearrange("p (j i n) -> p j i n", i=2, n=QCH)

            # ---- DMA programming: arrival order sets queue priority ----
            nc.sync.dma_start(xs_sb[:, :, 0:1024], xs_d.ap()[:, :, 0:1024])
            nc.sync.dma_start(rs_sb[:, :, ts(0, 1024)], rs_d.ap()[:, :, ts(0, 1024)])
            nc.sync.dma_start(rs_sb[:, :, ts(1, 1024)], rs_d.ap()[:, :, ts(1, 1024)])
            nc.scalar.dma_start(wb_sb[:], wb_d.ap())
            nc.scalar.dma_start(wf_sb[:], wf_d.ap())
            nc.scalar.dma_start(xs_sb[:, :, 1024:2048], xs_d.ap()[:, :, 1024:2048])
            nc.scalar.dma_start(rs_sb[:, :, ts(2, 1024)], rs_d.ap()[:, :, ts(2, 1024)])
            nc.scalar.dma_start(rs_sb[:, :, ts(3, 1024)], rs_d.ap()[:, :, ts(3, 1024)])
            nc.scalar.dma_start(resob_sb[:], resob_d.ap())
            nc.gpsimd.memset(wsrc[:], 1.0)
            nc.gpsimd.memset(ones32[:], 1.0)
            onesF = cpool.tile([128, 128], f32, tag="onesF")
            nc.gpsimd.memset(onesF[:], 1.0 / 32.0)
            nc.vector.tensor_copy(onesR[:], onesF[:])

            # warm the exp table while weights stream in
            warm = cpool.tile([128, 1], f32, tag="warm")
            nc.scalar.activation(warm[:], wsrc[:, :1], EXP)

            # ---- PE warm-up: trip the activity window during the DMA wait
            wps = ps_t.tile([128, 512], f32, tag="tps")
            for _ in range(NDUMMY):
                nc.tensor.matmul(wps[:, :128], wsrc[:], wsrc[:])

            # ---- QT = q_wT.T @ xs + q_b (pairs of 512-query chunks) ----
            def qt_pair(jj):
                qps = ps_s.tile([128, EB, QCH], f32, tag="sps", name="qps")
                for i in range(2):
                    for co in range(2):
                        nc.tensor.matmul(
                            qps[:, i, :],
                            qw[:, co, :],
                            xs_sb[:, co, ts(2 * jj + i, QCH)],
                            start=(co == 0),
                            stop=(co == 1),
                        )
                nc.scalar.activation(QT_r[:, jj, :, :], qps[:], IDENT, bias=qb)

            # ---- K = k_wT.T @ rs (pairs of 512-token chunks) ----
            def k_pair(jj):
                kps = ps_s.tile([128, EB, QCH], f32, tag="sps", name="kps")
                for i in range(2):
                    for co in range(2):
                        nc.tensor.matmul(
                            kps[:, i, :],
                            kw[:, co, :],
                            rs_sb[:, co, ts(2 * jj + i, QCH)],
                            start=(co == 0),
                            stop=(co == 1),
                        )
                nc.vector.tensor_copy(K_r[:, jj, :, :], kps[:])

            # ---- V^T quads: V_sb[:, kc, :] = rs_chunk.T @ v_wT directly ----
            # (Pool can't read PSUM, so casts alternate DVE / ACT-copy)
            def vt_quad(q, on_act):
                tps = ps_t.tile([128, 4, 128], f32, tag="tps", name="vtps")
                for i in range(4):
                    kc = 4 * q + i
                    for co in range(2):
                        nc.tensor.matmul(
                            tps[:, i, :],
                            rs_sb[:, co, ts(kc, 128)],
                            vw2[:, co, :],
                            start=(co == 0),
                            stop=(co == 1),
                        )
                if on_act:
                    nc.scalar.copy(V_sb[:, 4 * q : 4 * q + 4, :], tps[:])
                else:
                    nc.vector.tensor_copy(V_sb[:, 4 * q : 4 * q + 4, :], tps[:])

            # ---- main flash loop phases ----
            state = {}

            def s_phase(qc, qq):
                qsl = ts(qc, QCH)
                if qq == 0:
                    dps = ps_d.tile([128, QCH], f32, tag="dps", name="dps")
                    ops = ps_o.tile([128, QCH], f32, tag="ops", name="ops")
                    state[qc] = [dps, ops]
                et = bigpool.tile([128, KQ, QCH], bf16, tag="big")
                for bb in range(KQ // EB):
                    sps = ps_s.tile([128, EB, QCH], f32, tag="sps", name="sps")
                    for i in range(EB):
                        kc = qq * KQ + bb * EB + i
                        nc.tensor.matmul(
                            sps[:, i, :],
                            K_sb[:, ts(kc, 128)],
                            QT_sb[:, qsl],
                        )
                    if DVE_PAT[qq][bb]:
                        nc.vector.tensor_scalar(
                            et[:, ts(bb, EB), :].bitcast(i16),
                            sps[:],
                            SCH_A,
                            SCH_B,
                            MULT,
                            ADD,
                        )
                    else:
                        nc.scalar.activation(et[:, ts(bb, EB), :], sps[:], EXP)
                return et

            def avd_phase(qc, qq, et):
                dps, ops = state[qc][0], state[qc][1]
                for i in range(KQ):
                    kc = qq * KQ + i
                    nc.tensor.matmul(
                        ops[:],
                        V_sb[:, kc, :],
                        et[:, i, :],
                        start=(kc == 0),
                        stop=(kc == NKC - 1),
                        skip_group_check=True,
                    )
                # denominator partials: 4-way column-packed M=32 matmuls
                for i in range(KQ):
                    kc = qq * KQ + i
                    g = kc % 4
                    nc.tensor.matmul(
                        dps[32 * g : 32 * (g + 1), :],
                        ones32[:],
                        et[:, i, :],
                        start=(kc < 4),
                        stop=(kc >= NKC - 4),
                        skip_group_check=True,
                        tile_position=(0, 32 * g),
                    )
                if qq == NQUARTER - 1:
                    # free the accumulation banks right away (short DVE copies)
                    d_part = wpool.tile([128, QCH], f32r, tag="dpart")
                    nc.vector.tensor_copy(d_part[:], dps[:])
                    o_sb = wpool.tile([128, QCH], f32r, tag="osb")
                    nc.vector.tensor_copy(o_sb[:], ops[:])
                    state[qc] = [d_part, o_sb]

            def tail(qc, last=False):
                # project the UNNORMALIZED O^T; normalize after the matmul so
                # the PE never waits on the reciprocal.  For the last chunk,
                # split every elementwise stage in halves across DVE + Pool
                # and DMA each output half as soon as it is ready (this tail
                # is the end-to-end critical path).
                qsl = ts(qc, QCH)
                d_part, o_sb = state.pop(qc)
                tl = ps_o.tile([128, 512], f32, tag="ops", name="dfold")
                nc.tensor.matmul(tl[:], onesR[:], d_part[:])
                rec = wpool.tile([128, QCH], f32, tag="rec")
                nc.vector.reciprocal_approx_fast(rec[:], tl[:])
                ot = wpool.tile([128, 2, QCH], f32, tag="ost")
                for t in range(2):
                    pool_, tag_ = (ps_t, "tps") if t == 0 else (ps_s, "sps")
                    tf = pool_.tile([128, 512], f32, tag=tag_, name="tf")
                    nc.tensor.matmul(tf[:], ow[:, ts(t, 128)], o_sb[:])
                    tmp = wpool.tile([128, QCH], f32, tag="tmp", bufs=4)
                    if last:
                        h = QCH // 2
                        nc.vector.tensor_mul(tmp[:, :h], tf[:, :h], rec[:, :h])
                        nc.vector.tensor_mul(tmp[:, h:], tf[:, h:], rec[:, h:])
                        nc.vector.tensor_add(
                            ot[:, t, :h], tmp[:, :h], resob_sb[:, t, qsl][:, :h]
                        )
                        nc.gpsimd.tensor_add(
                            ot[:, t, h:], tmp[:, h:], resob_sb[:, t, qsl][:, h:]
                        )
                        nc.sync.dma_start(
                            out_d.ap()[:, t, qsl], ot[:, t, :]
                        )
                    else:
                        nc.vector.tensor_mul(tmp[:], tf[:], rec[:])
                        nc.gpsimd.tensor_add(
                            ot[:, t, :], tmp[:], resob_sb[:, t, qsl]
                        )
                if not last:
                    nc.sync.dma_start(out_d.ap()[:, :, qsl], ot[:])

            def bridge(n):
                for _ in range(n):
                    nc.tensor.matmul(wps[:, :128], wsrc[:], wsrc[:])

            # ---- emission schedule ----
            qt_pair(0)                      # queries 0:1024 (chunks 0,1)
            bridge(8)
            k_pair(0)                       # tokens 0:1024 (kc 0..7)
            bridge(4)
            et00 = s_phase(0, 0)
            vt_quad(0, False)
            vt_quad(1, True)
            k_pair(1)
            et01 = s_phase(0, 1)
            vt_quad(2, False)
            vt_quad(3, True)
            avd_phase(0, 0, et00)
            k_pair(2)
            et02 = s_phase(0, 2)
            vt_quad(4, False)
            vt_quad(5, True)
            avd_phase(0, 1, et01)
            k_pair(3)
            et03 = s_phase(0, 3)
            vt_quad(6, False)
            vt_quad(7, True)
            avd_phase(0, 2, et02)
            qt_pair(1)                      # queries 1024:2048 (chunks 2,3)
            et10 = s_phase(1, 0)
            avd_phase(0, 3, et03)

            pend = (1, 0, et10)
            tails = [0]
            jobs = [(qc, qq) for qc in range(NQC) for qq in range(NQUARTER)][5:]
            for qc, qq in jobs:
                et = s_phase(qc, qq)
                avd_phase(*pend)
                if pend[1] == NQUARTER - 1:
                    tails.append(pend[0])
                if tails and qq == 1:
                    tail(tails.pop(0))
                pend = (qc, qq, et)
            avd_phase(*pend)
            tails.append(pend[0])
            for t_ in tails:
                tail(t_, last=(t_ == NQC - 1))

    nc.compile()
    return nc


def _get_nc():
    if "nc" not in _CACHE:
        _CACHE["nc"] = _build_nc()
    return _CACHE["nc"]


def make_in_maps(rgbd, x, q_w, q_b, k_w, k_b, v_w, v_b, out_w, out_b):
    """Host-side sharding + weight swizzles. Returns per-core input maps."""
    import ml_dtypes

    f = np.float32
    bf = ml_dtypes.bfloat16
    rgbd = np.asarray(rgbd, f)
    x = np.asarray(x, f)
    q_w = np.asarray(q_w, f)
    q_b = np.asarray(q_b, f)
    k_w = np.asarray(k_w, f)
    v_w = np.asarray(v_w, f)
    out_w = np.asarray(out_w, f)
    out_b = np.asarray(out_b, f)
    v_b = np.asarray(v_b, f)

    # [ci_in, co, m] = w[m, co*128 + ci_in]  (projection weights, transposed)
    def swz(w):
        return w.reshape(CI, 2, 128).transpose(2, 1, 0)

    # v_w arranged for direct V^T: [c_in, co, ci_out] = v_w[ci_out, co*128+c_in]
    vw2 = v_w.T.reshape(2, 128, CI).transpose(1, 0, 2)
    wb = np.ascontiguousarray(
        np.concatenate([swz(q_w), swz(k_w), vw2], axis=2).astype(bf)
    )  # [128, 2, 384]

    ob_fused = out_b + out_w @ v_b                           # [C]
    wf = np.ascontiguousarray(
        np.concatenate([out_w.T, q_b.reshape(CI, 1)], axis=1)
    )  # [128, 257] f32

    # activations: [C, n] -> [128, 2, n] with c = co*128 + ci
    def actswz(a, dt=None):  # a: [C, n]
        sw = a.reshape(2, 128, -1).transpose(1, 0, 2)
        return np.ascontiguousarray(sw.astype(bf) if dt is None else sw.astype(dt))

    rs_all = rgbd.reshape(B, C, N)
    xs_all = x.reshape(B, C, N)
    resob_all = rgbd.reshape(B, C, N) + ob_fused[None, :, None]

    in_maps = []
    for core in range(NCORES):
        b, h = divmod(core, 2)
        sl = slice(h * QSH, (h + 1) * QSH)
        in_maps.append(
            {
                "rs": actswz(rs_all[b]),
                "xs": actswz(xs_all[b][:, sl]),
                "resob": actswz(resob_all[b][:, sl]),
                "wb": wb,
                "wf": wf,
            }
        )
    return in_maps


def gather_out(results):
    out = np.empty((B, C, N), np.float32)
    for core in range(NCORES):
        b, h = divmod(core, 2)
        o = results[core]["out"]  # [128, 2, QSH]
        out[b][:, h * QSH : (h + 1) * QSH] = o.transpose(1, 0, 2).reshape(C, QSH)
    return out.reshape(B, C, HH, WW)


def kernel(**inputs):
    from concourse.bass_utils import run_bass_kernel_spmd

    in_maps = make_in_maps(**inputs)
    nc = _get_nc()
    res = run_bass_kernel_spmd(nc, in_maps, list(range(NCORES)))
    return gather_out(res.results)


# revision 14
# speedup vs baseline: 1.0160x; 1.0160x over previous
"""Cross-attention kernel for Trainium2 (8 NeuronCores, Bass/Tile).

Problem: nn_CrossAttention — B=4, C=256, H=W=64 (N=4096 tokens), CI=128.
  q = q_w @ x + q_b            [B, N, CI]
  k = k_w @ rgbd + k_b         [B, CI, N]
  v = v_w @ rgbd + v_b         [B, N, CI]
  out = rgbd + out_w @ (softmax(q k) v) + out_b

Sharding: data-parallel over batch x query-half. Core i handles batch i//2,
query half i%2 (2048 queries, all 4096 keys). No collectives needed.

Math simplifications (exact):
  - k_b drops out of softmax (adds a per-query constant to logits).
  - v_b commutes with the softmax average -> fused output bias (host).
  - exp() without max-subtraction: logits are bounded (|S| < ~30), safe fp32.

Perf design (vs. the 133us baseline):
  - PE p-state: the PE reaches 2.4GHz only after ~3us of CONTINUOUS busy;
    any idle gap resets it to 1.2GHz.  Everything is arranged to keep the
    PE streaming without gaps: warm-up dummies bridge the input-DMA wait,
    and no consumer chain ever blocks the PE queue.
  - exp is split between ScalarE (ACT table exp) and VectorE (Schraudolph
    bitcast exp: et = bf16_bits(int16(A*s + B)), ~3% rel err that cancels
    in softmax since the denominator sums the same approximated values).
  - No ACT table swaps: ScalarE runs ONLY Exp/Identity (one table load).
    1/d uses DVE reciprocal_approx_fast (single custom-DVE op).
  - V^T computed directly (rs chunks as stationary weights) - no PE
    transposes, casts ride the idle GpSimd/Pool engine.
  - Host pre-swizzles inputs to the exact SBUF layouts so every DMA is a
    simple descriptor; DMAs are issued from 4 different engine queues in
    parallel right at kernel start.  Residual comes from the bf16 rs/resb
    copies (no separate f32 residual load).
  - Normalization is applied after the output projection, so the final
    matmuls never wait on the reciprocal; tail elementwise work is spread
    over Pool and DVE.
"""

import numpy as np

B, C, HH, WW = 4, 256, 64, 64
CI = 128
N = HH * WW            # 4096 tokens per batch
NCORES = 8
QSH = N // 2           # 2048 queries per core
QCH = 512              # query chunk (matmul moving free dim)
NQC = QSH // QCH       # 4 q-chunks
NKC = N // 128         # 32 key chunks of 128
EB = 2                 # S tiles per exp batch
KQ = 8                 # key chunks per et quarter-buffer
NQUARTER = NKC // KQ   # 4 quarters
NDUMMY = 46            # PE warm-up matmuls (128-free each)

# Schraudolph exp->bf16 constants: I = trunc(A*s + B) as int16, bitcast bf16.
# A = 128/ln2; B centers the log-error band (+0.5 for truncation).
SCH_A = 184.66502305344
SCH_B = 16250.9914
# per-quarter exp engine pattern (True -> DVE Schraudolph), 10 ACT / 6 DVE
DVE_PAT = [
    (False, True, False, True),
    (False, True, False, False),
    (False, True, False, False),
    (False, True, False, True),
]

_CACHE = {}


def _build_nc():
    import concourse.bass as bass
    import concourse.mybir as mybir
    import concourse.tile as tile
    from concourse import bacc
    from concourse.bass import ts

    f32 = mybir.dt.float32
    f32r = mybir.dt.float32r
    bf16 = mybir.dt.bfloat16
    i16 = mybir.dt.int16
    EXP = mybir.ActivationFunctionType.Exp
    IDENT = mybir.ActivationFunctionType.Identity
    ADD = mybir.AluOpType.add
    MULT = mybir.AluOpType.mult

    nc = bacc.Bacc("TRN2", target_bir_lowering=False, debug=False)

    rs_d = nc.dram_tensor("rs", [128, 2, N], bf16, kind="ExternalInput")
    xs_d = nc.dram_tensor("xs", [128, 2, QSH], bf16, kind="ExternalInput")
    resob_d = nc.dram_tensor("resob", [128, 2, QSH], bf16, kind="ExternalInput")
    wb_d = nc.dram_tensor("wb", [128, 2, 384], bf16, kind="ExternalInput")
    wf_d = nc.dram_tensor("wf", [128, 257], f32r, kind="ExternalInput")
    out_d = nc.dram_tensor("out", [128, 2, QSH], f32, kind="ExternalOutput")

    with tile.TileContext(nc) as tc:
        with (
            tc.tile_pool(name="const", bufs=1) as cpool,
            tc.tile_pool(name="big", bufs=3) as bigpool,
            tc.tile_pool(name="work", bufs=2) as wpool,
            tc.tile_pool(name="ps_s", bufs=2, space=bass.MemorySpace.PSUM) as ps_s,
            tc.tile_pool(name="ps_d", bufs=1, space=bass.MemorySpace.PSUM) as ps_d,
            tc.tile_pool(name="ps_o", bufs=2, space=bass.MemorySpace.PSUM) as ps_o,
            tc.tile_pool(name="ps_t", bufs=1, space=bass.MemorySpace.PSUM) as ps_t,
        ):
            # ---- SBUF tiles ----
            wb_sb = cpool.tile([128, 2, 384], bf16, tag="wb")
            wf_sb = cpool.tile([128, 257], f32r, tag="wf")
            rs_sb = cpool.tile([128, 2, N], bf16, tag="rs")
            xs_sb = cpool.tile([128, 2, QSH], bf16, tag="xs")
            resob_sb = cpool.tile([128, 2, QSH], bf16, tag="resob")
            K_sb = cpool.tile([128, N], bf16, tag="K")
            QT_sb = cpool.tile([128, QSH], bf16, tag="QT")
            V_sb = cpool.tile([128, NKC, 128], bf16, tag="V")
            ones32 = cpool.tile([128, 32], bf16, tag="ones32")
            onesR = cpool.tile([128, 128], f32r, tag="onesR")
            wsrc = cpool.tile([128, 128], bf16, tag="wsrc")

            qw = wb_sb[:, :, 0:128]
            kw = wb_sb[:, :, 128:256]
            vw2 = wb_sb[:, :, 256:384]
            ow = wf_sb[:, 0:256]
            qb = wf_sb[:, 256:257].bitcast(f32)

            # reshaped views for wide casts
            K_r = K_sb[:].rearrange("p (j i n) -> p j i n", i=2, n=QCH)
            QT_r = QT_sb[:].rearrange("p (j i n) -> p j i n", i=2, n=QCH)

            # ---- DMA programming: arrival order sets queue priority ----
            nc.sync.dma_start(xs_sb[:, :, 0:1024], xs_d.ap()[:, :, 0:1024])
            nc.sync.dma_start(rs_sb[:, :, ts(0, 1024)], rs_d.ap()[:, :, ts(0, 1024)])
            nc.sync.dma_start(rs_sb[:, :, ts(1, 1024)], rs_d.ap()[:, :, ts(1, 1024)])
            nc.scalar.dma_start(wb_sb[:], wb_d.ap())
            nc.scalar.dma_start(wf_sb[:], wf_d.ap())
            nc.scalar.dma_start(xs_sb[:, :, 1024:2048], xs_d.ap()[:, :, 1024:2048])
            nc.scalar.dma_start(rs_sb[:, :, ts(2, 1024)], rs_d.ap()[:, :, ts(2, 1024)])
            nc.scalar.dma_start(rs_sb[:, :, ts(3, 1024)], rs_d.ap()[:, :, ts(3, 1024)])
            nc.scalar.dma_start(resob_sb[:], resob_d.ap())
            nc.gpsimd.memset(wsrc[:], 1.0)
            nc.gpsimd.memset(ones32[:], 1.0)
            onesF = cpool.tile([128, 128], f32, tag="onesF")
            nc.gpsimd.memset(onesF[:], 1.0 / 32.0)
            nc.vector.tensor_copy(onesR[:], onesF[:])

            # warm the exp table while weights stream in
            warm = cpool.tile([128, 1], f32, tag="warm")
            nc.scalar.activation(warm[:], wsrc[:, :1], EXP)

            # ---- PE warm-up: trip the activity window during the DMA wait
            wps = ps_t.tile([128, 512], f32, tag="tps")
            for _ in range(NDUMMY):
                nc.tensor.matmul(wps[:, :128], wsrc[:], wsrc[:])

            # ---- QT = q_wT.T @ xs + q_b (pairs of 512-query chunks) ----
            def qt_pair(jj):
                qps = ps_s.tile([128, EB, QCH], f32, tag="sps", name="qps")
                for i in range(2):
                    for co in range(2):
                        nc.tensor.matmul(
                            qps[:, i, :],
                            qw[:, co, :],
                            xs_sb[:, co, ts(2 * jj + i, QCH)],
                            start=(co == 0),
                            stop=(co == 1),
                        )
                nc.scalar.activation(QT_r[:, jj, :, :], qps[:], IDENT, bias=qb)

            # ---- K = k_wT.T @ rs (pairs of 512-token chunks) ----
            def k_pair(jj, split=False):
                kps = ps_s.tile([128, EB, QCH], f32, tag="sps", name="kps")
                for i in range(2):
                    for co in range(2):
                        nc.tensor.matmul(
                            kps[:, i, :],
                            kw[:, co, :],
                            rs_sb[:, co, ts(2 * jj + i, QCH)],
                            start=(co == 0),
                            stop=(co == 1),
                        )
                if split:
                    nc.vector.tensor_copy(K_r[:, jj, 0, :], kps[:, 0, :])
                    nc.scalar.copy(K_r[:, jj, 1, :], kps[:, 1, :])
                else:
                    nc.vector.tensor_copy(K_r[:, jj, :, :], kps[:])

            # ---- V^T quads: V_sb[:, kc, :] = rs_chunk.T @ v_wT directly ----
            # (Pool can't read PSUM, so casts alternate DVE / ACT-copy)
            def vt_quad(q, on_act):
                tps = ps_t.tile([128, 4, 128], f32, tag="tps", name="vtps")
                for i in range(4):
                    kc = 4 * q + i
                    for co in range(2):
                        nc.tensor.matmul(
                            tps[:, i, :],
                            rs_sb[:, co, ts(kc, 128)],
                            vw2[:, co, :],
                            start=(co == 0),
                            stop=(co == 1),
                        )
                if on_act:
                    nc.scalar.copy(V_sb[:, 4 * q : 4 * q + 4, :], tps[:])
                else:
                    nc.vector.tensor_copy(V_sb[:, 4 * q : 4 * q + 4, :], tps[:])

            # ---- main flash loop phases ----
            state = {}

            def s_phase(qc, qq):
                qsl = ts(qc, QCH)
                if qq == 0:
                    dps = ps_d.tile([128, QCH], f32, tag="dps", name="dps")
                    ops = ps_o.tile([128, QCH], f32, tag="ops", name="ops")
                    state[qc] = [dps, ops]
                et = bigpool.tile([128, KQ, QCH], bf16, tag="big")
                for bb in range(KQ // EB):
                    sps = ps_s.tile([128, EB, QCH], f32, tag="sps", name="sps")
                    for i in range(EB):
                        kc = qq * KQ + bb * EB + i
                        nc.tensor.matmul(
                            sps[:, i, :],
                            K_sb[:, ts(kc, 128)],
                            QT_sb[:, qsl],
                        )
                    if DVE_PAT[qq][bb]:
                        nc.vector.tensor_scalar(
                            et[:, ts(bb, EB), :].bitcast(i16),
                            sps[:],
                            SCH_A,
                            SCH_B,
                            MULT,
                            ADD,
                        )
                    else:
                        nc.scalar.activation(et[:, ts(bb, EB), :], sps[:], EXP)
                return et

            def avd_phase(qc, qq, et, den_first=False):
                dps, ops = state[qc][0], state[qc][1]

                def av_mms():
                    for i in range(KQ):
                        kc = qq * KQ + i
                        nc.tensor.matmul(
                            ops[:],
                            V_sb[:, kc, :],
                            et[:, i, :],
                            start=(kc == 0),
                            stop=(kc == NKC - 1),
                            skip_group_check=True,
                        )

                def den_mms():
                    # denominator partials: 4-way column-packed M=32 matmuls
                    for i in range(KQ):
                        kc = qq * KQ + i
                        g = kc % 4
                        nc.tensor.matmul(
                            dps[32 * g : 32 * (g + 1), :],
                            ones32[:],
                            et[:, i, :],
                            start=(kc < 4),
                            stop=(kc >= NKC - 4),
                            skip_group_check=True,
                            tile_position=(0, 32 * g),
                        )

                last = qq == NQUARTER - 1
                d_part = o_sb = None
                if last:
                    d_part = wpool.tile([128, QCH], f32r, tag="dpart")
                    o_sb = wpool.tile([128, QCH], f32r, tag="osb")
                if den_first:
                    den_mms()
                    nc.vector.tensor_copy(d_part[:], dps[:])
                    av_mms()
                else:
                    av_mms()
                    den_mms()
                    if last:
                        nc.vector.tensor_copy(d_part[:], dps[:])
                if last:
                    # free the accumulation banks right away (short copies)
                    if den_first:
                        nc.vector.tensor_copy(o_sb[:, :256], ops[:, :256])
                        nc.scalar.copy(o_sb[:, 256:], ops[:, 256:])
                    else:
                        nc.vector.tensor_copy(o_sb[:], ops[:])
                    state[qc] = [d_part, o_sb]

            def tail(qc, last=False):
                # project the UNNORMALIZED O^T; normalize after the matmul so
                # the PE never waits on the reciprocal.  For the last chunk,
                # split every elementwise stage in halves across DVE + Pool
                # and DMA each output half as soon as it is ready (this tail
                # is the end-to-end critical path).
                qsl = ts(qc, QCH)
                d_part, o_sb = state.pop(qc)
                tl = ps_o.tile([128, 512], f32, tag="ops", name="dfold")
                nc.tensor.matmul(tl[:], onesR[:], d_part[:])
                rec = wpool.tile([128, QCH], f32, tag="rec")
                nc.vector.reciprocal_approx_fast(rec[:], tl[:])
                ot = wpool.tile([128, 2, QCH], f32, tag="ost")
                for t in range(2):
                    pool_, tag_ = (ps_t, "tps") if t == 0 else (ps_s, "sps")
                    tf = pool_.tile([128, 512], f32, tag=tag_, name="tf")
                    nc.tensor.matmul(tf[:], ow[:, ts(t, 128)], o_sb[:])
                    tmp = wpool.tile([128, QCH], f32, tag="tmp", bufs=4)
                    if last:
                        h = QCH // 2
                        nc.vector.tensor_mul(tmp[:, :h], tf[:, :h], rec[:, :h])
                        nc.vector.tensor_mul(tmp[:, h:], tf[:, h:], rec[:, h:])
                        nc.vector.tensor_add(
                            ot[:, t, :h], tmp[:, :h], resob_sb[:, t, qsl][:, :h]
                        )
                        nc.gpsimd.tensor_add(
                            ot[:, t, h:], tmp[:, h:], resob_sb[:, t, qsl][:, h:]
                        )
                        dma_eng = nc.sync if t == 0 else nc.scalar
                        dma_eng.dma_start(out_d.ap()[:, t, qsl], ot[:, t, :])
                    else:
                        nc.vector.tensor_mul(tmp[:], tf[:], rec[:])
                        nc.gpsimd.tensor_add(
                            ot[:, t, :], tmp[:], resob_sb[:, t, qsl]
                        )
                if not last:
                    nc.sync.dma_start(out_d.ap()[:, :, qsl], ot[:])

            def bridge(n):
                for _ in range(n):
                    nc.tensor.matmul(wps[:, :128], wsrc[:], wsrc[:])

            # ---- emission schedule ----
            qt_pair(0)                      # queries 0:1024 (chunks 0,1)
            bridge(8)
            k_pair(0, split=True)           # tokens 0:1024 (kc 0..7)
            bridge(12)
            et00 = s_phase(0, 0)
            bridge(4)
            vt_quad(0, False)
            vt_quad(1, True)
            bridge(4)
            k_pair(1)
            et01 = s_phase(0, 1)
            vt_quad(2, False)
            vt_quad(3, True)
            avd_phase(0, 0, et00)
            k_pair(2)
            et02 = s_phase(0, 2)
            vt_quad(4, False)
            vt_quad(5, True)
            avd_phase(0, 1, et01)
            k_pair(3)
            et03 = s_phase(0, 3)
            vt_quad(6, False)
            vt_quad(7, True)
            avd_phase(0, 2, et02)
            qt_pair(1)                      # queries 1024:2048 (chunks 2,3)
            et10 = s_phase(1, 0)
            avd_phase(0, 3, et03)

            pend = (1, 0, et10)
            tails = [0]
            jobs = [(qc, qq) for qc in range(NQC) for qq in range(NQUARTER)][5:]
            for qc, qq in jobs:
                et = s_phase(qc, qq)
                avd_phase(*pend)
                if pend[1] == NQUARTER - 1:
                    tails.append(pend[0])
                if tails and qq == 1:
                    tail(tails.pop(0))
                pend = (qc, qq, et)
            avd_phase(*pend, den_first=True)
            tails.append(pend[0])
            for t_ in tails:
                tail(t_, last=(t_ == NQC - 1))

    nc.compile()
    return nc


def _get_nc():
    if "nc" not in _CACHE:
        _CACHE["nc"] = _build_nc()
    return _CACHE["nc"]


def make_in_maps(rgbd, x, q_w, q_b, k_w, k_b, v_w, v_b, out_w, out_b):
    """Host-side sharding + weight swizzles. Returns per-core input maps."""
    import ml_dtypes

    f = np.float32
    bf = ml_dtypes.bfloat16
    rgbd = np.asarray(rgbd, f)
    x = np.asarray(x, f)
    q_w = np.asarray(q_w, f)
    q_b = np.asarray(q_b, f)
    k_w = np.asarray(k_w, f)
    v_w = np.asarray(v_w, f)
    out_w = np.asarray(out_w, f)
    out_b = np.asarray(out_b, f)
    v_b = np.asarray(v_b, f)

    # [ci_in, co, m] = w[m, co*128 + ci_in]  (projection weights, transposed)
    def swz(w):
        return w.reshape(CI, 2, 128).transpose(2, 1, 0)

    # v_w arranged for direct V^T: [c_in, co, ci_out] = v_w[ci_out, co*128+c_in]
    vw2 = v_w.T.reshape(2, 128, CI).transpose(1, 0, 2)
    wb = np.ascontiguousarray(
        np.concatenate([swz(q_w), swz(k_w), vw2], axis=2).astype(bf)
    )  # [128, 2, 384]

    ob_fused = out_b + out_w @ v_b                           # [C]
    wf = np.ascontiguousarray(
        np.concatenate([out_w.T, q_b.reshape(CI, 1)], axis=1)
    )  # [128, 257] f32

    # activations: [C, n] -> [128, 2, n] with c = co*128 + ci
    def actswz(a, dt=None):  # a: [C, n]
        sw = a.reshape(2, 128, -1).transpose(1, 0, 2)
        return np.ascontiguousarray(sw.astype(bf) if dt is None else sw.astype(dt))

    rs_all = rgbd.reshape(B, C, N)
    xs_all = x.reshape(B, C, N)
    resob_all = rgbd.reshape(B, C, N) + ob_fused[None, :, None]

    in_maps = []
    for core in range(NCORES):
        b, h = divmod(core, 2)
        sl = slice(h * QSH, (h + 1) * QSH)
        in_maps.append(
            {
                "rs": actswz(rs_all[b]),
                "xs": actswz(xs_all[b][:, sl]),
                "resob": actswz(resob_all[b][:, sl]),
                "wb": wb,
                "wf": wf,
            }
        )
    return in_maps


def gather_out(results):
    out = np.empty((B, C, N), np.float32)
    for core in range(NCORES):
        b, h = divmod(core, 2)
        o = results[core]["out"]  # [128, 2, QSH]
        out[b][:, h * QSH : (h + 1) * QSH] = o.transpose(1, 0, 2).reshape(C, QSH)
    return out.reshape(B, C, HH, WW)


def kernel(**inputs):
    from concourse.bass_utils import run_bass_kernel_spmd

    in_maps = make_in_maps(**inputs)
    nc = _get_nc()
    res = run_bass_kernel_spmd(nc, in_maps, list(range(NCORES)))
    return gather_out(res.results)
